# revision 1
# baseline (speedup 1.0000x reference)
"""Trainium2 Bass kernel for nn_MoELayer_71038759076630.

Top-2-of-8 MoE with a SwiGLU shared expert: out = shared(x) + sum_k g_k * E_k(x).

Strategy (8 NeuronCores, SPMD single program):
  - Expert-parallel: core c owns expert c. The host computes the router
    (fp64), gathers each expert's tokens into a capacity-padded batch
    [D, C] (C = max expert load rounded up to 128), and scatters gated
    results back.
  - The shared expert is token-parallel: core c processes tokens
    [512c, 512(c+1)).
  - On device everything is feature-major (tokens on the free axis):
        stage 1: uT[h] = sum_d w1T[d,h].T @ xT[d,:] (PSUM accum), silu*mul
        stage 2: oT[d] = sum_h w2T[h,d].T @ hT[h,:] (PSUM accum) -> DRAM
    All matmuls run as float32r (full PE rate at moving dim >= 256).
  - Expert token batches are split into parts <= 640 so the full-H hidden
    activations stay resident in SBUF; weights stream tile-contiguously
    (host pre-packs them in PE tile order).
  - DMA queues: w1/w3 stream on SP, w2 + outputs on ACT (one-ahead
    prefetch), token inputs on gpsimd.

The harness-facing entry point is kernel(**inputs) -> np.ndarray.
"""

from contextlib import ExitStack

import numpy as np

import concourse.bass as bass
import concourse.mybir as mybir
import concourse.tile as tile
from concourse.bass_utils import run_bass_kernel_spmd

FP32 = mybir.dt.float32
FP32R = mybir.dt.float32r
SILU = mybir.ActivationFunctionType.Silu

P = 128
B, S, D = 2, 2048, 2048
E, K = 8, 2
H = (D * 4) // 3          # 2730
HP = 2816                 # H padded to 22 * 128
N_CORES = 8
HT = HP // P              # 22 h tiles
HH = HT // 2              # 11 per half
DT = D // P               # 16 d tiles


# ----------------------------------------------------------------------------
# device program
# ----------------------------------------------------------------------------

def token_chunks(n):
    """Split n (multiple of 128, >=256) into chunks <=512 (one PSUM bank)
    and >=256 (fp32r full-rate threshold)."""
    out = []
    while n > 768:
        out.append(512)
        n -= 512
    if n > 512:
        out += [n - 256, 256]
    else:
        out.append(n)
    return out


def token_parts(C):
    """Split C into phase-sized parts (<=640 so full-H hT fits SBUF),
    smallest first (quicker first-phase ramp)."""
    parts = []
    while C > 768:
        parts.append(640)
        C -= 640
    if C > 640:
        parts += [C - 256, 256]
    else:
        parts.append(C)
    return sorted(parts)


def split_waits(nc, max_waits=1):
    """walrus in this toolchain accepts at most one sync-wait per
    instruction; hoist extras onto preceding NoOps on the same engine."""
    nsplit = 0
    for f in nc.m.functions:
        for bb in f.blocks:
            new_insts = []
            for inst in bb.instructions:
                si = getattr(inst, "sync_info", None)
                if si is not None and si.on_wait and len(si.on_wait) > max_waits:
                    waits = list(si.on_wait)
                    extra, keep = waits[:-max_waits], waits[-max_waits:]
                    for i, w in enumerate(extra):
                        nop = mybir.InstNoOp(
                            name=f"{inst.name}_ws{i}",
                            engine=inst.engine, ins=[], outs=[],
                            sync_info=mybir.SyncInfo(on_wait=[w], on_update=[]),
                        )
                        new_insts.append(nop)
                        nsplit += 1
                    si.on_wait = keep
                new_insts.append(inst)
            bb.instructions[:] = new_insts
    return nsplit


def emit_x_dmas(nc, engine, xt, xin_dram, t0, n_tok, group=4):
    """Load token columns [t0, t0+n_tok) into tile xt, one DMA per
    `group` d_tiles (few issues, still races consumption)."""
    for di in range(0, DT, group):
        g = min(group, DT - di)
        engine.dma_start(
            xt[:, di:di + g, :],
            xin_dram[di * P:(di + g) * P, t0:t0 + n_tok].rearrange(
                "(n p) t -> p n t", p=P))


def _phase(nc, pools, xt, n_tok, w1t, w3t, w2t, out_dram, out_t0):
    """Full-H SwiGLU MLP over n_tok tokens; writes out_dram[:, out_t0:+n_tok]."""
    p_w13, p_w2, p_h, p_stg, ps_uv, ps_o = pools
    chs = token_chunks(n_tok)

    ht = [p_h.tile([P, n_tok], FP32R, tag="h", name=f"ht{i}") for i in range(HT)]

    # ---- stage 1 (w1/w3 stream on SP, slot-paced prefetch) ----
    wq = []

    def emit_w13(hi):
        w1b = p_w13.tile([P, DT, P], FP32R, tag="w13", name=f"w1b{hi}")
        nc.sync.dma_start(w1b[:], w1t[hi].rearrange("p (n h) -> p n h", h=P))
        w3b = p_w13.tile([P, DT, P], FP32R, tag="w13", name=f"w3b{hi}")
        nc.sync.dma_start(w3b[:], w3t[hi].rearrange("p (n h) -> p n h", h=P))
        wq.append((w1b, w3b))

    emit_w13(0)
    emit_w13(1)
    for hi in range(HT):
        if hi + 2 < HT:
            emit_w13(hi + 2)
        w1b, w3b = wq[hi]
        t0 = 0
        for ch in chs:
            u_ps = ps_uv.tile([P, ch], FP32, tag="uv", name="u_ps")
            v_ps = ps_uv.tile([P, ch], FP32, tag="uv", name="v_ps")
            for di in range(DT):
                nc.tensor.matmul(
                    u_ps[:], w1b[:, di, :], xt[:, di, t0:t0 + ch],
                    start=(di == 0), stop=(di == DT - 1),
                )
            for di in range(DT):
                nc.tensor.matmul(
                    v_ps[:], w3b[:, di, :], xt[:, di, t0:t0 + ch],
                    start=(di == 0), stop=(di == DT - 1),
                )
            su = p_stg.tile([P, ch], FP32, tag="stg", name="su")
            nc.scalar.activation(su[:], u_ps[:], SILU)
            nc.vector.tensor_mul(ht[hi][:, t0:t0 + ch], su[:], v_ps[:])
            t0 += ch

    # ---- stage 2 (w2 stream one-ahead on ACT queue) ----
    w2q = []

    def emit_w2(di):
        w2b0 = p_w2.tile([P, HH, P], FP32R, tag="w2", name=f"w2a{di}")
        nc.scalar.dma_start(w2b0[:], w2t[0, di].rearrange("p (n d) -> p n d", d=P))
        w2b1 = p_w2.tile([P, HH, P], FP32R, tag="w2", name=f"w2b{di}")
        nc.scalar.dma_start(w2b1[:], w2t[1, di].rearrange("p (n d) -> p n d", d=P))
        w2q.append((w2b0, w2b1))

    emit_w2(0)
    for di in range(DT):
        if di + 1 < DT:
            emit_w2(di + 1)
        w2b0, w2b1 = w2q[di]
        d0 = di * P
        t0 = 0
        for ch in chs:
            o_ps = ps_o.tile([P, ch], FP32, tag="o", name="o_ps")
            for hi in range(HT):
                wb = w2b0 if hi < HH else w2b1
                nc.tensor.matmul(
                    o_ps[:], wb[:, hi % HH, :], ht[hi][:, t0:t0 + ch],
                    start=(hi == 0), stop=(hi == HT - 1),
                )
            og = p_stg.tile([P, ch], FP32, tag="stg", name="og")
            nc.vector.tensor_copy(og[:], o_ps[:])
            nc.scalar.dma_start(
                out_dram[d0:d0 + P, out_t0 + t0:out_t0 + t0 + ch], og[:])
            t0 += ch


def build_moe_nc(C, n_shared_tok):
    """One SPMD program run on all 8 cores (per-core data differs)."""
    nc = bass.Bass()
    xs = nc.dram_tensor("xs", [D, n_shared_tok], FP32R, kind="ExternalInput")
    xe = nc.dram_tensor("xe", [D, C], FP32R, kind="ExternalInput")
    w1t = nc.dram_tensor("w1t", [HT, P, D], FP32R, kind="ExternalInput")
    w3t = nc.dram_tensor("w3t", [HT, P, D], FP32R, kind="ExternalInput")
    w2t = nc.dram_tensor("w2t", [2, DT, P, HH * P], FP32R, kind="ExternalInput")
    sw1t = nc.dram_tensor("sw1t", [HT, P, D], FP32R, kind="ExternalInput")
    sw3t = nc.dram_tensor("sw3t", [HT, P, D], FP32R, kind="ExternalInput")
    sw2t = nc.dram_tensor("sw2t", [2, DT, P, HH * P], FP32R, kind="ExternalInput")
    ys = nc.dram_tensor("ys", [D, n_shared_tok], FP32, kind="ExternalOutput")
    ye = nc.dram_tensor("ye", [D, C], FP32, kind="ExternalOutput")

    parts = token_parts(C)
    max_part = max([n_shared_tok] + parts)

    with tile.TileContext(nc) as tc, ExitStack() as ctx:
        p_w13 = ctx.enter_context(tc.tile_pool(name="w13", bufs=4))
        p_w2 = ctx.enter_context(tc.tile_pool(name="w2", bufs=3))
        p_stg = ctx.enter_context(tc.tile_pool(name="stg", bufs=2))
        p_h = ctx.enter_context(tc.tile_pool(name="h", bufs=23))
        p_xin = ctx.enter_context(tc.tile_pool(name="xin", bufs=2))
        ps_uv = ctx.enter_context(tc.tile_pool(name="uv", bufs=6, space="PSUM"))
        ps_o = ctx.enter_context(tc.tile_pool(name="o", bufs=2, space="PSUM"))
        pools = (p_w13, p_w2, p_h, p_stg, ps_uv, ps_o)

        # first expert part ramps the kernel: its input load (gpsimd) runs
        # in parallel with the weight stream (SP) from t=0
        def expert_phase(part, t0):
            xet = p_xin.tile([P, DT, max_part], FP32R, tag="xin", name="xet")
            emit_x_dmas(nc, nc.gpsimd, xet[:, :, :part], xe, t0, part)
            _phase(nc, pools, xet[:, :, :part], part, w1t, w3t, w2t, ye, t0)

        expert_phase(parts[0], 0)

        # shared phase; xs prefetches on gpsimd during the first phase
        xst = p_xin.tile([P, DT, max_part], FP32R, tag="xin", name="xst")
        emit_x_dmas(nc, nc.gpsimd, xst[:, :, :n_shared_tok], xs, 0, n_shared_tok)
        _phase(nc, pools, xst[:, :, :n_shared_tok], n_shared_tok,
               sw1t, sw3t, sw2t, ys, 0)

        t0 = parts[0]
        for part in parts[1:]:
            expert_phase(part, t0)
            t0 += part

    split_waits(nc)
    return nc


# ----------------------------------------------------------------------------
# host side
# ----------------------------------------------------------------------------

def _route(x_flat, wr):
    """fp64 router: softmax over logits, top-2, renormalized gates."""
    logits = x_flat.astype(np.float64) @ wr.astype(np.float64).T
    m = logits.max(-1, keepdims=True)
    p = np.exp(logits - m)
    p /= p.sum(-1, keepdims=True)
    top2 = np.argsort(-p, axis=-1)[:, :K]
    tp = np.take_along_axis(p, top2, -1)
    g = tp / tp.sum(-1, keepdims=True)
    return top2, g.astype(np.float32)


def _prep_w13(w):
    """[H, D] -> [HT, P, DT*P] in PE tile order (contiguous DMA)."""
    wp = np.zeros((HP, D), np.float32)
    wp[:H] = w
    return np.ascontiguousarray(
        wp.reshape(HT, P, DT, P).transpose(0, 3, 2, 1).reshape(HT, P, DT * P))


def _prep_w2(w):
    """[D, H] -> [2, DT, P, HH*P] in PE tile order (contiguous DMA)."""
    wp = np.zeros((D, HP), np.float32)
    wp[:, :H] = w
    return np.ascontiguousarray(
        wp.reshape(DT, P, 2, HH, P).transpose(2, 0, 4, 3, 1).reshape(
            2, DT, P, HH * P))


_NC_CACHE = {}


def kernel(x, sw1, sw2, sw3, ew1, ew2, ew3, wr):
    x = np.asarray(x, np.float32)
    sw1 = np.asarray(sw1, np.float32)
    sw2 = np.asarray(sw2, np.float32)
    sw3 = np.asarray(sw3, np.float32)
    ew1 = np.asarray(ew1, np.float32)
    ew2 = np.asarray(ew2, np.float32)
    ew3 = np.asarray(ew3, np.float32)
    wr = np.asarray(wr, np.float32)

    T = x.shape[0] * x.shape[1]
    x_flat = np.ascontiguousarray(x.reshape(T, D))
    top2, g = _route(x_flat, wr)

    idx_list, gate_list = [], []
    for e in range(E):
        sel = np.nonzero(top2 == e)
        idx_list.append(sel[0])
        gate_list.append(g[sel])
    cnt = [len(ix) for ix in idx_list]
    C = max(256, -(-max(cnt) // P) * P)
    ts = T // N_CORES

    sw1t, sw3t, sw2t = _prep_w13(sw1), _prep_w13(sw3), _prep_w2(sw2)
    in_maps = []
    for c in range(N_CORES):
        ix = idx_list[c]
        xe = np.zeros((D, C), np.float32)
        xe[:, :cnt[c]] = x_flat[ix].T
        xs = np.ascontiguousarray(x_flat[c * ts:(c + 1) * ts].T)
        in_maps.append({
            "xs": xs, "xe": xe,
            "w1t": _prep_w13(ew1[c]), "w3t": _prep_w13(ew3[c]),
            "w2t": _prep_w2(ew2[c]),
            "sw1t": sw1t, "sw3t": sw3t, "sw2t": sw2t,
        })

    key = (C, ts)
    if key not in _NC_CACHE:
        _NC_CACHE[key] = build_moe_nc(C, ts)
    nc = _NC_CACHE[key]
    res = run_bass_kernel_spmd(nc, in_maps, list(range(N_CORES)))

    out = np.empty((T, D), np.float32)
    for c in range(N_CORES):
        out[c * ts:(c + 1) * ts] = res.results[c]["ys"].T
    for c in range(N_CORES):
        ye = res.results[c]["ye"].T
        ix = idx_list[c]
        out[ix] += gate_list[c][:, None] * ye[:cnt[c]]
    return out.reshape(x.shape)
